# revision 8
# baseline (speedup 1.0000x reference)
"""Trainium2 Bass kernel for nn_DilatedGCN (gnn_message_passing).

Math (derived from the reference):
  feats F = X @ W_mlp + b_mlp                  [N, B, T, D]
  scores = concat([F[src], F[dst]]) @ W_attn + b_attn
  Per-destination-segment softmax over the DEG=8 incoming edges.
  The dst-side term is constant within a segment, so it cancels in the
  softmax; max-subtraction is unnecessary in f32.  Hence with
     S  = F @ W_attn[:D]        (per node)
     ES = exp(S)/8              (per node; /8 keeps fp8e4 in range and
                                 cancels in num/den)
     G  = ES * F                (per node)
  each dilation graph k is a segment-sum over incoming edges:
     gcn_k[n] = (sum_j G[src_j]) / (sum_j ES[src_j])
  out = leaky_relu(sum_k w_k * gcn_k, 0.01) + X

Key idea vs the gather-based variant: dst = repeat(arange(N), 8), so
gather+segment-sum == dense matmul with the (tiny-valued, exact-in-fp8)
edge-count matrix A_k[dst, src]:
     [den | num] = A_k @ [ES | G]
A DMA row-gather is HBM-latency-bound (~8.3 ns/row -> ~400 us for 48k
rows); the dense matmul replaces it with fp8 DoubleRow TensorE work
(~60 us) plus 12 MB of sequential A-tile loads that overlap compute.

Distribution: data-parallel over the 48 (b, t) pairs -> 6 per core.
Per core: 96 small MLP matmuls write the node tables H = [ES | G]
(fp8, [128, 16 s-blocks, 768]) directly into SBUF; per (k, dst-tile)
8+8 DoubleRow matmuls (contraction 256/instr) produce den/num in PSUM;
Vector does recip + weighted accumulation; final Lrelu + residual via
CCE-accumulate DMA.
"""

import numpy as np

B, N, T, C, D, K, DEG = 4, 2000, 12, 64, 64, 3, 8
E = N * DEG
NCORES = 8
BT = B * T              # 48
SPC = BT // NCORES      # 6 (b,t) slots per core
M = SPC * D             # 384 channels per node per core
NSB = 16                # node s-blocks of 128 (2000 -> 2048 padded)
NP = 128 * NSB          # 2048 padded nodes
NCH = NSB * SPC         # 96 MLP chunks of 128 nodes x 1 slot
LN8 = float(np.log(8.0))

_CACHE = {}


def _build_program(kstage=None, ksub=None, rep_all=1):
    import os
    import concourse.bacc as bacc
    import concourse.mybir as mybir
    from concourse.tile import TileContext
    from contextlib import ExitStack

    if kstage is None:
        kstage = os.environ.get("KSTAGE", "3")
    if ksub is None:
        ksub = os.environ.get("KSUB", "gme")

    dt = mybir.dt
    nc = bacc.Bacc("TRN2")

    xT1 = nc.dram_tensor("xT1", [C + 1, NCH * 128], dt.bfloat16,
                         kind="ExternalInput")
    w2cat = nc.dram_tensor("w2cat", [C + 1, 2 * D], dt.bfloat16,
                           kind="ExternalInput")
    # A^T tiles: atiles[k, p, ot*16*128 + s*128 + c] = #edges(k, dst=128*ot+c,
    # src=128*s+p); counts <= 8 are exact in fp8e4
    atiles = nc.dram_tensor("atiles", [K, 128, NSB * NSB * 128], dt.float8e4,
                            kind="ExternalInput")
    wkcol = nc.dram_tensor("wkcol", [128, K], dt.float32, kind="ExternalInput")
    x_rows = nc.dram_tensor("x_rows", [N, M], dt.float32, kind="ExternalInput")
    out_rows = nc.dram_tensor("out_rows", [N, M], dt.float32,
                              kind="ExternalOutput")

    with TileContext(nc) as tc, ExitStack() as ctx:
        from concourse.library_config import mlp
        nc.gpsimd.load_library(mlp)
        const = ctx.enter_context(tc.tile_pool(name="const", bufs=1))
        sc = ctx.enter_context(tc.tile_pool(name="scratch", bufs=4))
        ep = ctx.enter_context(tc.tile_pool(name="epi", bufs=3))
        pps = ctx.enter_context(tc.tile_pool(name="prolps", bufs=2,
                                             space="PSUM"))
        rps = ctx.enter_context(tc.tile_pool(name="redps", bufs=6,
                                             space="PSUM"))

        for _rep in range(rep_all):
            _kernel_body(nc, tc, dt, mybir, kstage, ksub,
                         const, sc, ep, pps, rps,
                         xT1, w2cat, atiles, wkcol, x_rows, out_rows)

    nc.compile()
    return nc


def _kernel_body(nc, tc, dt, mybir, KSTAGE, KSUB,
                 const, sc, ep, pps, rps,
                 xT1, w2cat, atiles, wkcol, x_rows, out_rows):
    AF = mybir.ActivationFunctionType
    ALU = mybir.AluOpType
    DR = mybir.MatmulPerfMode.DoubleRow

    # ---------------- loads ----------------
    bias_t = sc.tile([128, 1], dt.float32, tag="bias")
    nc.gpsimd.memset(bias_t[:], -LN8)
    w2_sb = sc.tile([C + 1, 2 * D], dt.bfloat16, tag="w2")
    nc.scalar.dma_start(w2_sb[:], w2cat[:])
    wk_sb = sc.tile([128, K], dt.float32, tag="wk")
    nc.scalar.dma_start(wk_sb[:], wkcol[:])
    xsb = const.tile([C + 1, NCH * 128], dt.bfloat16)
    for q in range(4):
        w = NCH * 128 // 4
        nc.sync.dma_start(xsb[:, q * w:(q + 1) * w], xT1[:, q * w:(q + 1) * w])

    # A^T slab: one 4 MB DMA per graph, contiguous per partition
    at_all = const.tile([128, K, NSB, NSB, 128], dt.float8e4)
    if "g" in KSUB:
        for k in range(K):
            nc.sync.dma_start(
                at_all[:, k].rearrange("p o s c -> p (o s c)"), atiles[k])
    else:
        nc.gpsimd.memset(at_all[:], 0.125)

    acc = const.tile([128, NSB * M], dt.float32)
    # node tables: H[p, s, 0:384] = ES, H[p, s, 384:768] = G  (node 128s+p)
    H = const.tile([128, NSB, 2 * M], dt.float8e4)

    # ---------------- prologue: MLP -> H in SBUF ----------------
    for s in range(NSB):
        for t in range(SPC):
            ci = s * SPC + t
            ps = pps.tile([128, 2 * D], dt.float32, tag="pp")
            nc.tensor.matmul(out=ps[:], lhsT=xsb[:, 128 * ci:128 * (ci + 1)],
                             rhs=w2_sb[:], start=True, stop=True)
            es_t = sc.tile([128, D], dt.bfloat16, tag="es")
            nc.scalar.activation(es_t[:], ps[:, D:], AF.Exp, bias=bias_t[:])
            f_t = sc.tile([128, D], dt.bfloat16, tag="f")
            nc.vector.tensor_copy(f_t[:], ps[:, :D])
            nc.vector.tensor_copy(H[:, s, D * t:D * (t + 1)], es_t[:])
            nc.vector.tensor_tensor(H[:, s, M + D * t:M + D * (t + 1)],
                                    f_t[:], es_t[:], op=ALU.mult)

    # ---------------- main: dense A^T @ [ES|G] per graph ----------------
    for k in range(K if KSTAGE >= "1" else 0):
        for ot in range(NSB):
            if "m" not in KSUB:
                continue
            denp = rps.tile([128, M], dt.float32, tag="red")
            nump = rps.tile([128, M], dt.float32, tag="red")
            for s2 in range(NSB // 2):
                lt = at_all[:, k, ot, 2 * s2:2 * s2 + 2, :]
                nc.tensor.matmul(out=denp[:], lhsT=lt,
                                 rhs=H[:, 2 * s2:2 * s2 + 2, :M],
                                 start=(s2 == 0), stop=(s2 == NSB // 2 - 1),
                                 perf_mode=DR)
                nc.tensor.matmul(out=nump[:], lhsT=lt,
                                 rhs=H[:, 2 * s2:2 * s2 + 2, M:],
                                 start=(s2 == 0), stop=(s2 == NSB // 2 - 1),
                                 perf_mode=DR)
            if "e" not in KSUB:
                continue
            pv = 128 if ot < NSB - 1 else N - 128 * (NSB - 1)
            rt = ep.tile([128, M], dt.float32, tag="rt")
            nc.vector.reciprocal_approx_fast(out=rt[:pv, :], in_=denp[:pv, :])
            accv = acc[:pv, M * ot:M * (ot + 1)]
            if k == 0:
                nc.vector.scalar_tensor_tensor(
                    accv, rt[:pv, :], wk_sb[:pv, k:k + 1], nump[:pv, :],
                    op0=ALU.mult, op1=ALU.mult)
            else:
                tmp = ep.tile([128, M], dt.float32, tag="tmp")
                nc.vector.scalar_tensor_tensor(
                    tmp[:pv, :], rt[:pv, :], wk_sb[:pv, k:k + 1],
                    nump[:pv, :], op0=ALU.mult, op1=ALU.mult)
                nc.vector.tensor_tensor(accv, accv, tmp[:pv, :], op=ALU.add)

    # ---------------- final: leaky relu + residual + store ----------------
    for slot in range(NSB if KSTAGE >= "2" else 0):
        pv = 128 if slot < NSB - 1 else N - 128 * (NSB - 1)
        ot = ep.tile([128, M], dt.float32, tag="out")
        accv = acc[:pv, M * slot:M * (slot + 1)]
        # leaky_relu(x, .01) = .01*x + relu(.99*x)
        rl = ep.tile([128, M], dt.float32, tag="rl")
        nc.scalar.activation(rl[:pv, :], accv, AF.Relu, scale=0.99)
        nc.vector.scalar_tensor_tensor(ot[:pv, :], accv, 0.01, rl[:pv, :],
                                       op0=ALU.mult, op1=ALU.add)
        base = 128 * slot
        if KSTAGE >= "3":
            # residual: ot += x_rows rows (CCE accumulate into SBUF)
            nc.gpsimd.dma_start(ot[:pv, :], x_rows[base:base + pv, :],
                                accum_op=ALU.add)
        nc.sync.dma_start(out_rows[base:base + pv, :], ot[:pv, :])


def _get_program(kstage=None, ksub=None, rep_all=1):
    key = ("nc", kstage, ksub, rep_all)
    if key not in _CACHE:
        _CACHE[key] = _build_program(kstage, ksub, rep_all)
    return _CACHE[key]


def _prep_inputs(input_feature, W_mlp, b_mlp, W_attn, b_attn, weight, edges):
    import ml_dtypes
    bf16 = ml_dtypes.bfloat16
    fp8 = ml_dtypes.float8_e4m3

    X = np.asarray(input_feature, dtype=np.float32)
    src = np.asarray(edges)[:, 0, :].astype(np.int64)
    dst = np.asarray(edges)[:, 1, :].astype(np.int64)
    assert src.min() >= 0 and src.max() < N
    assert dst.min() >= 0 and dst.max() < N

    A65 = np.concatenate([np.asarray(W_mlp, np.float32),
                          np.asarray(b_mlp, np.float32)[None, :]], axis=0)
    Wa = np.asarray(W_attn, np.float32)[:D, :]
    w2cat_h = np.ascontiguousarray(
        np.concatenate([A65, A65 @ Wa], axis=1).astype(bf16))  # [65, 128]

    # edge-count tiles: at_h[k, p, ot, s, c] = #edges(k, dst=128ot+c, src=128s+p)
    counts = np.zeros((K, NP, NP), np.uint8)
    kk = np.repeat(np.arange(K), E)
    np.add.at(counts, (kk, src.reshape(-1), dst.reshape(-1)), 1)
    at_h = np.ascontiguousarray(
        counts.reshape(K, NSB, 128, NSB, 128).transpose(0, 2, 3, 1, 4)
        .reshape(K, 128, NSB * NSB * 128).astype(fp8))

    wk = np.asarray(weight, np.float32).reshape(K)
    wkcol_h = np.ascontiguousarray(
        np.broadcast_to(wk[None, :], (128, K)).astype(np.float32))

    # per-core slices: slot = b*T + t; core c owns slots [6c, 6c+6)
    Xn = np.transpose(X, (1, 0, 2, 3)).reshape(N, BT, C)
    in_maps = []
    for c in range(NCORES):
        Xloc = Xn[:, SPC * c:SPC * (c + 1), :]                   # [N, 6, C]
        x_rows_h = np.ascontiguousarray(Xloc.reshape(N, M))
        Xpad = np.zeros((NP, SPC, C), np.float32)
        Xpad[:N] = Xloc
        xT1_h = np.empty((C + 1, NCH * 128), dtype=bf16)
        # col (s*SPC+t)*128 + i -> node 128s+i, slot t
        xT1_h[:C] = (Xpad.reshape(NSB, 128, SPC, C)
                     .transpose(3, 0, 2, 1).reshape(C, NCH * 128).astype(bf16))
        xT1_h[C] = np.asarray(1.0, dtype=bf16)
        in_maps.append({
            "xT1": np.ascontiguousarray(xT1_h),
            "w2cat": w2cat_h,
            "atiles": at_h,
            "wkcol": wkcol_h,
            "x_rows": x_rows_h,
        })
    return in_maps


def _assemble_output(results):
    out_all = np.empty((N, BT, C), dtype=np.float32)
    for c in range(NCORES):
        out_all[:, SPC * c:SPC * (c + 1), :] = \
            results[c]["out_rows"].reshape(N, SPC, C)
    return np.ascontiguousarray(
        out_all.reshape(N, B, T, C).transpose(1, 0, 2, 3))


def kernel(input_feature, W_mlp, b_mlp, W_attn, b_attn, weight, edges,
           _trace=False, **trace_kwargs):
    from concourse.bass_utils import run_bass_kernel_spmd

    in_maps = _prep_inputs(input_feature, W_mlp, b_mlp, W_attn, b_attn,
                           weight, edges)
    nc = _get_program()
    res = run_bass_kernel_spmd(nc, in_maps, list(range(NCORES)),
                               trace=_trace, **trace_kwargs)
    out = _assemble_output(res.results)
    if _trace:
        return out, res
    return out


# revision 15
# speedup vs baseline: 166.9851x; 166.9851x over previous
"""Trainium2 Bass kernel for nn_DilatedGCN (gnn_message_passing).

Math (derived from the reference):
  feats F = X @ W_mlp + b_mlp                  [N, B, T, D]
  scores = concat([F[src], F[dst]]) @ W_attn + b_attn
  Per-destination-segment softmax over the DEG=8 incoming edges.
  The dst-side term is constant within a segment, so it cancels in the
  softmax; max-subtraction is unnecessary in f32.  Hence with
     S  = F @ W_attn[:D]        (per node)
     ES = exp(S)/8              (per node; /8 keeps fp8e4 in range and
                                 cancels in num/den)
     G  = ES * F                (per node)
  each dilation graph k is a segment-sum over incoming edges:
     gcn_k[n] = (sum_j G[src_j]) / (sum_j ES[src_j])
  out = leaky_relu(sum_k w_k * gcn_k, 0.01) + X

Key idea vs the gather-based variant: dst = repeat(arange(N), 8), so
gather+segment-sum == dense matmul with the (tiny-valued, exact-in-fp8)
edge-count matrix A_k[dst, src]:
     [den | num] = A_k @ [ES | G]
A DMA row-gather is HBM-latency-bound (~8.3 ns/row -> ~400 us for 48k
rows); the dense matmul replaces it with fp8 DoubleRow TensorE work
(~60 us) plus 12 MB of sequential A-tile loads that overlap compute.

Distribution: data-parallel over the 48 (b, t) pairs -> 6 per core.
Per core: 96 small MLP matmuls write the node tables H = [ES | G]
(fp8, [128, 16 s-blocks, 768]) directly into SBUF; per (k, dst-tile)
8+8 DoubleRow matmuls (contraction 256/instr) produce den/num in PSUM;
Vector does recip + weighted accumulation; final Lrelu + residual via
CCE-accumulate DMA.
"""

import os as _os
import shutil as _shutil

import numpy as np

# The libneuronxla on-disk NEFF cache can key different bass programs to the
# same fingerprint across processes (the hash covers in-process identity that
# repeats across deterministic startups), silently loading a stale NEFF from
# an earlier session. One recompile is cheap; a wrong NEFF is not.
_shutil.rmtree(_os.path.expanduser("~/.neuron-compile-cache"),
               ignore_errors=True)

B, N, T, C, D, K, DEG = 4, 2000, 12, 64, 64, 3, 8
E = N * DEG
NCORES = 8
BT = B * T              # 48
SPC = BT // NCORES      # 6 (b,t) slots per core
M = SPC * D             # 384 channels per node per core
NSB = 16                # node s-blocks of 128 (2000 -> 2048 padded)
NP = 128 * NSB          # 2048 padded nodes
NCH = NSB * SPC         # 96 MLP chunks of 128 nodes x 1 slot
LN8 = float(np.log(8.0))

_CACHE = {}


def _build_program(kstage=None, ksub=None, rep_all=1):
    import os
    import concourse.bacc as bacc
    import concourse.mybir as mybir
    from concourse.tile import TileContext
    from contextlib import ExitStack

    if kstage is None:
        kstage = os.environ.get("KSTAGE", "3")
    if ksub is None:
        ksub = os.environ.get("KSUB", "gme")

    dt = mybir.dt
    nc = bacc.Bacc("TRN2")

    xT1 = nc.dram_tensor("xT1", [C + 1, NCH * 128], dt.bfloat16,
                         kind="ExternalInput")
    w2cat = nc.dram_tensor("w2cat", [C + 1, 2 * D], dt.bfloat16,
                           kind="ExternalInput")
    # A^T tiles: atiles[ot, p, (k*16+s)*128 + c] = #edges(k, dst=128*ot+c,
    # src=128*s+p); counts <= 8 are exact in fp8e4.  ot-major so the first
    # dst-tile's lhsT data (all 3 graphs) lands in SBUF within ~2 us.
    atiles = nc.dram_tensor("atiles", [NSB, 128, K * NSB * 128], dt.float8e4,
                            kind="ExternalInput")
    wkcol = nc.dram_tensor("wkcol", [128, K], dt.float32, kind="ExternalInput")
    x_rows = nc.dram_tensor("x_rows", [N, M], dt.float32, kind="ExternalInput")
    out_rows = nc.dram_tensor("out_rows", [N, M], dt.float32,
                              kind="ExternalOutput")

    with TileContext(nc) as tc, ExitStack() as ctx:
        from concourse.library_config import mlp
        nc.gpsimd.load_library(mlp)
        const = ctx.enter_context(tc.tile_pool(name="const", bufs=1))
        sc = ctx.enter_context(tc.tile_pool(name="scratch", bufs=4))
        ep = ctx.enter_context(tc.tile_pool(name="epi", bufs=3))
        pps = ctx.enter_context(tc.tile_pool(name="prolps", bufs=2,
                                             space="PSUM"))
        rps = ctx.enter_context(tc.tile_pool(name="redps", bufs=6,
                                             space="PSUM"))

        for _rep in range(rep_all):
            _kernel_body(nc, tc, dt, mybir, kstage, ksub,
                         const, sc, ep, pps, rps,
                         xT1, w2cat, atiles, wkcol, x_rows, out_rows)

    nc.compile()
    return nc


def _kernel_body(nc, tc, dt, mybir, KSTAGE, KSUB,
                 const, sc, ep, pps, rps,
                 xT1, w2cat, atiles, wkcol, x_rows, out_rows):
    AF = mybir.ActivationFunctionType
    ALU = mybir.AluOpType
    DR = mybir.MatmulPerfMode.DoubleRow

    # ---------------- loads ----------------
    bias_t = sc.tile([128, 1], dt.float32, tag="bias")
    nc.gpsimd.memset(bias_t[:], -LN8)
    w2_sb = sc.tile([C + 1, 2 * D], dt.bfloat16, tag="w2")
    nc.scalar.dma_start(w2_sb[:], w2cat[:])
    wk_sb = sc.tile([128, K], dt.float32, tag="wk")
    nc.scalar.dma_start(wk_sb[:], wkcol[:])
    xsb = const.tile([C + 1, NCH * 128], dt.bfloat16)
    for q in range(4):
        w = NCH * 128 // 4
        nc.sync.dma_start(xsb[:, q * w:(q + 1) * w], xT1[:, q * w:(q + 1) * w])

    # A^T slab: one 768 KB DMA per dst-tile (covers all 3 graphs)
    at_all = const.tile([128, NSB, K, NSB, 128], dt.float8e4)
    if "g" in KSUB:
        for ot in range(NSB):
            nc.sync.dma_start(
                at_all[:, ot].rearrange("p k s c -> p (k s c)"), atiles[ot])
    else:
        nc.gpsimd.memset(at_all[:], 0.125)

    # node tables: H[p, s, 0:384] = ES, H[p, s, 384:768] = G  (node 128s+p)
    H = const.tile([128, NSB, 2 * M], dt.float8e4)

    # ---------------- prologue: MLP -> H in SBUF ----------------
    for s in range(NSB):
        for t in range(SPC):
            ci = s * SPC + t
            ps = pps.tile([128, 2 * D], dt.float32, tag="pp")
            nc.tensor.matmul(out=ps[:], lhsT=xsb[:, 128 * ci:128 * (ci + 1)],
                             rhs=w2_sb[:], start=True, stop=True)
            esv = H[:, s, D * t:D * (t + 1)]
            nc.scalar.activation(esv, ps[:, D:], AF.Exp, bias=bias_t[:])
            nc.vector.tensor_tensor(H[:, s, M + D * t:M + D * (t + 1)],
                                    ps[:, :D], esv, op=ALU.mult)

    # ------------- main: [den|num] = A_k^T @ [ES|G], fused epilogue -------
    for ot in range(NSB if KSTAGE >= "1" else 0):
        pv = 128 if ot < NSB - 1 else N - 128 * (NSB - 1)
        accv = None
        for k in range(K):
            if "m" not in KSUB:
                continue
            denp = rps.tile([128, M], dt.float32, tag="red")
            nump = rps.tile([128, M], dt.float32, tag="red")
            for s2 in range(NSB // 2):
                lt = at_all[:, ot, k, 2 * s2:2 * s2 + 2, :]
                nc.tensor.matmul(out=denp[:], lhsT=lt,
                                 rhs=H[:, 2 * s2:2 * s2 + 2, :M],
                                 start=(s2 == 0), stop=(s2 == NSB // 2 - 1),
                                 perf_mode=DR)
                nc.tensor.matmul(out=nump[:], lhsT=lt,
                                 rhs=H[:, 2 * s2:2 * s2 + 2, M:],
                                 start=(s2 == 0), stop=(s2 == NSB // 2 - 1),
                                 perf_mode=DR)
            if "e" not in KSUB:
                continue
            rt = ep.tile([128, M], dt.float32, tag="rt")
            nc.vector.reciprocal_approx_fast(out=rt[:pv, :],
                                             in_=denp[:pv, :])
            if k == 0:
                acct = ep.tile([128, M], dt.float32, tag="acc")
                nc.vector.scalar_tensor_tensor(
                    acct[:pv, :], rt[:pv, :], wk_sb[:pv, k:k + 1],
                    nump[:pv, :], op0=ALU.mult, op1=ALU.mult)
                accv = acct
            else:
                tmp = ep.tile([128, M], dt.float32, tag="tmp")
                nc.vector.scalar_tensor_tensor(
                    tmp[:pv, :], rt[:pv, :], wk_sb[:pv, k:k + 1],
                    nump[:pv, :], op0=ALU.mult, op1=ALU.mult)
                nc.vector.tensor_tensor(accv[:pv, :], accv[:pv, :],
                                        tmp[:pv, :], op=ALU.add)
        if KSTAGE < "2" or accv is None:
            continue
        # final: leaky_relu(x, .01) = .01*x + relu(.99*x), then residual
        ott = ep.tile([128, M], dt.float32, tag="out")
        rl = ep.tile([128, M], dt.float32, tag="rl")
        nc.scalar.activation(rl[:pv, :], accv[:pv, :], AF.Relu, scale=0.99)
        nc.vector.scalar_tensor_tensor(ott[:pv, :], accv[:pv, :], 0.01,
                                       rl[:pv, :], op0=ALU.mult, op1=ALU.add)
        base = 128 * ot
        if KSTAGE >= "3":
            # residual: ott += x_rows rows (CCE accumulate into SBUF)
            nc.gpsimd.dma_start(ott[:pv, :], x_rows[base:base + pv, :],
                                accum_op=ALU.add)
        nc.sync.dma_start(out_rows[base:base + pv, :], ott[:pv, :])


def _get_program(kstage=None, ksub=None, rep_all=1):
    key = ("nc", kstage, ksub, rep_all)
    if key not in _CACHE:
        _CACHE[key] = _build_program(kstage, ksub, rep_all)
    return _CACHE[key]


def _prep_inputs(input_feature, W_mlp, b_mlp, W_attn, b_attn, weight, edges):
    import ml_dtypes
    bf16 = ml_dtypes.bfloat16
    fp8 = ml_dtypes.float8_e4m3

    X = np.asarray(input_feature, dtype=np.float32)
    src = np.asarray(edges)[:, 0, :].astype(np.int64)
    dst = np.asarray(edges)[:, 1, :].astype(np.int64)
    assert src.min() >= 0 and src.max() < N
    assert dst.min() >= 0 and dst.max() < N

    A65 = np.concatenate([np.asarray(W_mlp, np.float32),
                          np.asarray(b_mlp, np.float32)[None, :]], axis=0)
    Wa = np.asarray(W_attn, np.float32)[:D, :]
    w2cat_h = np.ascontiguousarray(
        np.concatenate([A65, A65 @ Wa], axis=1).astype(bf16))  # [65, 128]

    # edge-count tiles: at_h[ot, p, k, s, c] = #edges(k, dst=128ot+c, src=128s+p)
    counts = np.zeros((K, NP, NP), np.uint8)
    kk = np.repeat(np.arange(K), E)
    np.add.at(counts, (kk, src.reshape(-1), dst.reshape(-1)), 1)
    at_h = np.ascontiguousarray(
        counts.reshape(K, NSB, 128, NSB, 128).transpose(3, 2, 0, 1, 4)
        .reshape(NSB, 128, K * NSB * 128).astype(fp8))

    wk = np.asarray(weight, np.float32).reshape(K)
    wkcol_h = np.ascontiguousarray(
        np.broadcast_to(wk[None, :], (128, K)).astype(np.float32))

    # per-core slices: slot = b*T + t; core c owns slots [6c, 6c+6)
    Xn = np.transpose(X, (1, 0, 2, 3)).reshape(N, BT, C)
    in_maps = []
    for c in range(NCORES):
        Xloc = Xn[:, SPC * c:SPC * (c + 1), :]                   # [N, 6, C]
        x_rows_h = np.ascontiguousarray(Xloc.reshape(N, M))
        Xpad = np.zeros((NP, SPC, C), np.float32)
        Xpad[:N] = Xloc
        xT1_h = np.empty((C + 1, NCH * 128), dtype=bf16)
        # col (s*SPC+t)*128 + i -> node 128s+i, slot t
        xT1_h[:C] = (Xpad.reshape(NSB, 128, SPC, C)
                     .transpose(3, 0, 2, 1).reshape(C, NCH * 128).astype(bf16))
        xT1_h[C] = np.asarray(1.0, dtype=bf16)
        in_maps.append({
            "xT1": np.ascontiguousarray(xT1_h),
            "w2cat": w2cat_h,
            "atiles": at_h,
            "wkcol": wkcol_h,
            "x_rows": x_rows_h,
        })
    return in_maps


def _assemble_output(results):
    out_all = np.empty((N, BT, C), dtype=np.float32)
    for c in range(NCORES):
        out_all[:, SPC * c:SPC * (c + 1), :] = \
            results[c]["out_rows"].reshape(N, SPC, C)
    return np.ascontiguousarray(
        out_all.reshape(N, B, T, C).transpose(1, 0, 2, 3))


def kernel(input_feature, W_mlp, b_mlp, W_attn, b_attn, weight, edges,
           _trace=False, **trace_kwargs):
    from concourse.bass_utils import run_bass_kernel_spmd

    in_maps = _prep_inputs(input_feature, W_mlp, b_mlp, W_attn, b_attn,
                           weight, edges)
    nc = _get_program()
    res = run_bass_kernel_spmd(nc, in_maps, list(range(NCORES)),
                               trace=_trace, **trace_kwargs)
    out = _assemble_output(res.results)
    if _trace:
        return out, res
    return out
